# revision 17
# baseline (speedup 1.0000x reference)
"""Trainium2 Bass kernel for nn_MultiHeadAttention (B=4, S=2048, E=512, H=8).

Returns (out, attn) matching the reference:
    Q = q @ Wq.T + bq ... scores = QK^T/sqrt(D) + position_bias, causal+key mask,
    attn = softmax(scores), ctx = attn @ V, out = ctx @ Wo.T + bo.

Distribution over 8 NeuronCores, two SPMD launches:
  launch 1 (row-parallel): QKV projections; each core handles 1024 of the
      8192 flattened (B*S) rows, writing Q^T/K^T (head-major transposed) and
      V (natural) layouts.
  launch 2 (head-parallel): core h computes head h for all batches: scores
      are built transposed (k on partitions, q free) so softmax sums come
      free out of a V-augmented matmul and attn@V needs no transposes; the
      natural-layout attention output is produced by PE transposes; the
      output projection partial (summed over heads on host) is also done
      on device.

All matmuls run in float32r (TF32-like, ~1.5e-4 rel err; full PE rate).
"""

import numpy as np

import concourse.bacc as bacc
import concourse.mybir as mybir
import concourse.tile as tile
from concourse.bass_utils import run_bass_kernel_spmd
from concourse.masks import make_identity

F32 = mybir.dt.float32
F32R = mybir.dt.float32r
EXP = mybir.ActivationFunctionType.Exp
IDENT = mybir.ActivationFunctionType.Identity

B, S, E, H = 4, 2048, 512, 8
D = E // H          # 64
SB = B * S          # 8192 flattened rows
R = SB // 8         # 1024 rows per core (launch 1)
NCORES = 8

TRACE = False
LAST_EXEC_NS = {}

_cache = {}


# ---------------------------------------------------------------- launch 1
def _build_proj():
    nc = bacc.Bacc("TRN2", target_bir_lowering=False, debug=False,
                   num_devices=NCORES)
    xq = nc.declare_dram_parameter("xq", [R, E], F32R, isOutput=False)
    xk = nc.declare_dram_parameter("xk", [R, E], F32R, isOutput=False)
    xv = nc.declare_dram_parameter("xv", [R, E], F32R, isOutput=False)
    wqt = nc.declare_dram_parameter("wqt", [E, E], F32R, isOutput=False)
    wkt = nc.declare_dram_parameter("wkt", [E, E], F32R, isOutput=False)
    wvt = nc.declare_dram_parameter("wvt", [E, E], F32R, isOutput=False)
    bq2 = nc.declare_dram_parameter("bq2", [128, 4], F32, isOutput=False)
    bk2 = nc.declare_dram_parameter("bk2", [128, 4], F32, isOutput=False)
    bvrow = nc.declare_dram_parameter("bvrow", [1, E], F32R, isOutput=False)
    qt_out = nc.declare_dram_parameter("qt_out", [E, R], F32, isOutput=True)
    kt_out = nc.declare_dram_parameter("kt_out", [E, R], F32, isOutput=True)
    v_out = nc.declare_dram_parameter("v_out", [R, E], F32, isOutput=True)

    NT = R // 128   # 8 row tiles
    NE = E // 128   # 4 emb chunks

    with tile.TileContext(nc) as tc:
        with tc.tile_pool(name="const", bufs=1) as const, \
             tc.tile_pool(name="w", bufs=1) as wpool, \
             tc.tile_pool(name="xt", bufs=1) as xtp, \
             tc.tile_pool(name="ld", bufs=3) as ld, \
             tc.tile_pool(name="st", bufs=3) as st, \
             tc.tile_pool(name="tp", bufs=3, space="PSUM") as tps, \
             tc.tile_pool(name="mp", bufs=3, space="PSUM") as mps:
            ident = const.tile([128, 128], F32, tag="ident")
            make_identity(nc, ident)
            ident_r = const.tile([128, 128], F32R, tag="ident_r")
            nc.vector.tensor_copy(ident_r[:], ident[:])
            ones_f = const.tile([1, 128], F32, tag="ones_f")
            nc.vector.memset(ones_f[:], 1.0)
            ones_r = const.tile([1, 128], F32R, tag="ones_r")
            nc.vector.tensor_copy(ones_r[:], ones_f[:])
            bq_sb = const.tile([128, 4], F32, tag="bq")
            bk_sb = const.tile([128, 4], F32, tag="bk")
            bv_sb = const.tile([1, E], F32R, tag="bv")
            nc.sync.dma_start(bq_sb[:], bq2[:])
            nc.sync.dma_start(bk_sb[:], bk2[:])
            nc.sync.dma_start(bv_sb[:], bvrow[:])

            ws = {}
            for name, wt in (("q", wqt), ("k", wkt), ("v", wvt)):
                ws[name] = [wpool.tile([128, E], F32R, tag=f"w{name}{i}",
                                       name=f"w{name}{i}")
                            for i in range(NE)]
                for i in range(NE):
                    nc.sync.dma_start(ws[name][i][:], wt[128 * i:128 * (i + 1), :])

            # transposed inputs: xT[name][e] is (128, R) slab of x^T
            xT = {}
            for name, x in (("q", xq), ("k", xk), ("v", xv)):
                xT[name] = [xtp.tile([128, R], F32R, tag=f"xT{name}{e}",
                                     name=f"xT{name}{e}")
                            for e in range(NE)]
                for t in range(NT):
                    xt_ld = ld.tile([128, E], F32R, tag="xld")
                    nc.sync.dma_start(xt_ld[:], x[128 * t:128 * (t + 1), :])
                    for e in range(NE):
                        pt = tps.tile([128, 128], F32R, tag="tp")
                        nc.tensor.matmul(pt[:], xt_ld[:, 128 * e:128 * (e + 1)],
                                         ident_r[:], is_transpose=True,
                                         start=True, stop=True)
                        nc.vector.tensor_copy(
                            xT[name][e][:, 128 * t:128 * (t + 1)], pt[:])

            # Q^T / K^T : out[e_out block, row chunk] = sum_e_in w[e_in][:, e_out].T @ xT[e_in]
            for name, out_t, bias_sb in (("q", qt_out, bq_sb), ("k", kt_out, bk_sb)):
                for eo in range(NE):
                    for rt in range(R // 512):
                        ps = mps.tile([128, 512], F32, tag="mm")
                        for ei in range(NE):
                            nc.tensor.matmul(
                                ps[:],
                                ws[name][ei][:, 128 * eo:128 * (eo + 1)],
                                xT[name][ei][:, 512 * rt:512 * (rt + 1)],
                                start=(ei == 0), stop=(ei == NE - 1))
                        ot = st.tile([128, 512], F32, tag="qkst")
                        nc.scalar.activation(ot[:], ps[:], IDENT,
                                             bias=bias_sb[:, eo:eo + 1], scale=1.0)
                        nc.sync.dma_start(
                            out_t[128 * eo:128 * (eo + 1), 512 * rt:512 * (rt + 1)],
                            ot[:])

            # V natural: out[row tile, :] = x[rows] @ wvt + bv
            for t in range(NT):
                ps = mps.tile([128, 512], F32, tag="mm")
                for ei in range(NE):
                    nc.tensor.matmul(ps[:], xT["v"][ei][:, 128 * t:128 * (t + 1)],
                                     ws["v"][ei][:], start=(ei == 0), stop=False)
                nc.tensor.matmul(ps[:], ones_r[:], bv_sb[:], start=False, stop=True)
                ot = st.tile([128, 512], F32, tag="vst")
                nc.vector.tensor_copy(ot[:], ps[:])
                nc.sync.dma_start(v_out[128 * t:128 * (t + 1), :], ot[:])
    nc.compile()
    return nc


# ---------------------------------------------------------------- launch 2
def _build_attn():
    import os
    dbg_nc = int(os.environ.get("KER_NC", "4"))
    dbg_nb = int(os.environ.get("KER_NB", "4"))
    dbg_noattn = os.environ.get("KER_NOATTN", "0") == "1"
    dbg_nozero = os.environ.get("KER_NOZERO", "0") == "1"
    dbg_nonat = os.environ.get("KER_NONAT", "0") == "1"
    nc = bacc.Bacc("TRN2", target_bir_lowering=False, debug=False,
                   num_devices=NCORES)
    qt = nc.declare_dram_parameter("qt", [D, SB], F32R, isOutput=False)
    kt = nc.declare_dram_parameter("kt", [D, SB], F32R, isOutput=False)
    # V padded to 128 cols: [:, :64] = V_h, [:, 64] = 1 (row sums), rest 0.
    # fp32r matmuls need all 128 PE col groups active (output partitions).
    vaug = nc.declare_dram_parameter("vaug", [SB, 128], F32R, isOutput=False)
    biasT = nc.declare_dram_parameter("biasT", [S, S], F32R, isOutput=False)
    wot = nc.declare_dram_parameter("wot", [D, E], F32R, isOutput=False)
    pen = nc.declare_dram_parameter("pen", [128, 64], F32, isOutput=False)
    maskt = nc.declare_dram_parameter("maskt", [128, 2048], F32, isOutput=False)
    attn_out = nc.declare_dram_parameter("attn_out", [B, S, S], F32, isOutput=True)
    outp = nc.declare_dram_parameter("outp", [B, S, E], F32, isOutput=True)

    NKT = S // 128      # 16 k tiles per batch
    NC_ = S // 512      # 4 q chunks per batch

    with tile.TileContext(nc) as tc:
        with tc.tile_pool(name="const", bufs=1) as const, \
             tc.tile_pool(name="bias", bufs=1) as pb, \
             tc.tile_pool(name="expp", bufs=1) as ep, \
             tc.tile_pool(name="astage", bufs=1) as ap_, \
             tc.tile_pool(name="ostage", bufs=2) as op_, \
             tc.tile_pool(name="small", bufs=2) as sp_, \
             tc.tile_pool(name="scp", bufs=2, space="PSUM") as scp, \
             tc.tile_pool(name="ctxp", bufs=2, space="PSUM") as ctxp, \
             tc.tile_pool(name="natp", bufs=2, space="PSUM") as natp_, \
             tc.tile_pool(name="opp", bufs=1, space="PSUM") as opp_, \
             tc.tile_pool(name="sump", bufs=1, space="PSUM") as sump:
            ident = const.tile([128, 128], F32, tag="ident")
            make_identity(nc, ident)
            ident_r = const.tile([128, 128], F32R, tag="ident_r")
            nc.vector.tensor_copy(ident_r[:], ident[:])
            zero_sb = const.tile([128, 512], F32, tag="zero")
            nc.vector.memset(zero_sb[:], 0.0)

            qt_sb = const.tile([D, SB], F32R, tag="qt")
            kt_sb = const.tile([D, SB], F32R, tag="kt")
            for i in range(4):
                # single (64, 8192) DMAs crash the device; chunk to (64, 2048)
                nc.sync.dma_start(qt_sb[:, 2048 * i:2048 * (i + 1)],
                                  qt[:, 2048 * i:2048 * (i + 1)])
                nc.sync.dma_start(kt_sb[:, 2048 * i:2048 * (i + 1)],
                                  kt[:, 2048 * i:2048 * (i + 1)])
            vaug_sb = const.tile([128, (SB // 128) * 128], F32R, tag="vaug")
            for t in range(SB // 128):
                nc.sync.dma_start(vaug_sb[:, 128 * t:128 * (t + 1)],
                                  vaug[128 * t:128 * (t + 1), :])
            wot_sb = const.tile([D, E], F32R, tag="wot")
            nc.sync.dma_start(wot_sb[:], wot[:])
            pen_sb = const.tile([128, 64], F32, tag="pen")
            nc.sync.dma_start(pen_sb[:], pen[:])
            maskt_sb = const.tile([128, 2048], F32, tag="maskt")
            nc.sync.dma_start(maskt_sb[:], maskt[:])

            for c in range(dbg_nc):
                jn = 4 * c + 4   # k tiles in this chunk column
                # bias^T tiles for this q-chunk, shared across batches
                bts = []
                for j in range(jn):
                    bt = pb.tile([128, 512], F32R, tag=f"bias{j}",
                                 name=f"bias{j}_{c}")
                    nc.sync.dma_start(
                        bt[:], biasT[128 * j:128 * (j + 1), 512 * c:512 * (c + 1)])
                    bts.append(bt)
                for b in range(dbg_nb):
                    ctps = ctxp.tile([128, 512], F32, tag="ctx")
                    expT = []
                    for j in range(jn):
                        sc = scp.tile([128, 512], F32, tag="sc")
                        nc.tensor.matmul(
                            sc[:],
                            kt_sb[:, S * b + 128 * j:S * b + 128 * (j + 1)],
                            qt_sb[:, S * b + 512 * c:S * b + 512 * (c + 1)],
                            start=True, stop=False)
                        nc.tensor.matmul(sc[:], ident_r[:], bts[j][:],
                                         start=False, stop=True,
                                         skip_group_check=True)
                        e = ep.tile([128, 512], F32R, tag=f"expT{j}")
                        nc.scalar.activation(e[:], sc[:], EXP,
                                             bias=pen_sb[:, 4 * j + b:4 * j + b + 1],
                                             scale=1.0)
                        if j >= 4 * c:
                            nc.vector.tensor_mul(
                                e[:], e[:],
                                maskt_sb[:, 512 * (j - 4 * c):512 * (j - 4 * c + 1)])
                        nc.tensor.matmul(
                            ctps[:],
                            vaug_sb[:, (16 * b + j) * 128:(16 * b + j + 1) * 128],
                            e[:], start=(j == 0), stop=(j == jn - 1),
                            skip_group_check=True)
                        expT.append(e)
                    # evict ctx^T rows [0..64) for the output projection,
                    # and rows [64..66) (sums + junk) shifted to partition 0
                    # for the 2-row PE transpose (fp32r needs an even count;
                    # non-zero base-partition transposes misbehave).
                    cts = sp_.tile([D, 512], F32R, tag="cts")
                    nc.vector.tensor_copy(cts[:], ctps[0:D, :])
                    s2 = sp_.tile([2, 512], F32R, tag="s2")
                    nc.vector.tensor_copy(s2[:], ctps[D:D + 2, :])
                    scol = sump.tile([128, 8], F32R, tag="scol")
                    for qm in range(4):
                        nc.tensor.matmul(scol[:, 2 * qm:2 * qm + 2],
                                         s2[:, 128 * qm:128 * (qm + 1)],
                                         ident_r[0:2, 0:2],
                                         is_transpose=True,
                                         start=True, stop=True)
                    rc = sp_.tile([128, 8], F32, tag="rc")
                    nc.vector.reciprocal(rc[:], scol[:])
                    for qm in range(4):
                        # output projection partial (normalized at evict)
                        op = opp_.tile([128, 512], F32, tag="op")
                        nc.tensor.matmul(op[:],
                                         cts[:, 128 * qm:128 * (qm + 1)],
                                         wot_sb[:], start=True, stop=True)
                        os = op_.tile([128, 512], F32, tag="os")
                        nc.scalar.mul(os[:], op[:], rc[:, 2 * qm:2 * qm + 1])
                        r0 = 512 * c + 128 * qm
                        nc.sync.dma_start(outp[b, r0:r0 + 128, :], os[:])
                        if dbg_noattn:
                            continue
                        # natural-layout attention rows
                        ast = ap_.tile([128, 512 * (c + 1)], F32, tag="astage")
                        if not dbg_nonat:
                            for jg in range(c + 1):
                                np_t = natp_.tile([128, 512], F32R, tag="nat")
                                for jj in range(4):
                                    j = 4 * jg + jj
                                    nc.tensor.matmul(
                                        np_t[:, 128 * jj:128 * (jj + 1)],
                                        expT[j][:, 128 * qm:128 * (qm + 1)],
                                        ident_r[:], is_transpose=True,
                                        start=True, stop=True)
                                nc.vector.tensor_scalar_mul(
                                    ast[:, 512 * jg:512 * (jg + 1)], np_t[:],
                                    rc[:, 2 * qm:2 * qm + 1])
                        else:
                            nc.vector.memset(ast[:], 0.5)
                        nc.sync.dma_start(
                            attn_out[b, r0:r0 + 128, 0:512 * (c + 1)], ast[:])
                        if not dbg_nozero:
                            for z in range(3 - c):
                                z0 = 512 * (c + 1) + 512 * z
                                nc.sync.dma_start(
                                    attn_out[b, r0:r0 + 128, z0:z0 + 512], zero_sb[:])
    nc.compile()
    return nc


# ---------------------------------------------------------------- host glue
def _get(name, builder):
    if name not in _cache:
        _cache[name] = builder()
    return _cache[name]


def _run(nc, in_maps, label):
    res = run_bass_kernel_spmd(nc, in_maps, core_ids=list(range(NCORES)),
                               trace=TRACE)
    if TRACE:
        LAST_EXEC_NS[label] = res.exec_time_ns
    return res.results


def _pad_vaug(v):
    out = np.zeros((SB, 128), np.float32)
    out[:, :D] = v
    out[:, D] = 1.0
    return out


def kernel(query, key, value, mask, position_bias,
           Wq, bq, Wk, bk, Wv, bv, Wo, bo):
    query = np.asarray(query, np.float32).reshape(SB, E)
    key_ = np.asarray(key, np.float32).reshape(SB, E)
    value = np.asarray(value, np.float32).reshape(SB, E)
    mask = np.asarray(mask)
    position_bias = np.asarray(position_bias, np.float32)
    Wq = np.asarray(Wq, np.float32); bq = np.asarray(bq, np.float32)
    Wk = np.asarray(Wk, np.float32); bk = np.asarray(bk, np.float32)
    Wv = np.asarray(Wv, np.float32); bv = np.asarray(bv, np.float32)
    Wo = np.asarray(Wo, np.float32); bo = np.asarray(bo, np.float32)

    scale = 1.0 / np.sqrt(D).astype(np.float32)

    # ---- launch 1: projections, row-sharded
    wqt = np.ascontiguousarray(Wq.T * scale)
    wkt = np.ascontiguousarray(Wk.T)
    wvt = np.ascontiguousarray(Wv.T)
    bq2 = np.ascontiguousarray((bq * scale).reshape(4, 128).T)
    bk2 = np.ascontiguousarray(bk.reshape(4, 128).T)
    bvrow = np.ascontiguousarray(bv.reshape(1, E))
    in1 = []
    for c in range(NCORES):
        r0 = R * c
        in1.append({
            "xq": query[r0:r0 + R], "xk": key_[r0:r0 + R], "xv": value[r0:r0 + R],
            "wqt": wqt, "wkt": wkt, "wvt": wvt,
            "bq2": bq2, "bk2": bk2, "bvrow": bvrow,
        })
    res1 = _run(_get("proj", _build_proj), in1, "proj")

    QT = np.empty((E, SB), np.float32)
    KT = np.empty((E, SB), np.float32)
    V = np.empty((SB, E), np.float32)
    for c in range(NCORES):
        QT[:, R * c:R * (c + 1)] = res1[c]["qt_out"]
        KT[:, R * c:R * (c + 1)] = res1[c]["kt_out"]
        V[R * c:R * (c + 1)] = res1[c]["v_out"]

    # ---- launch 2: attention, head-parallel
    penalty = ((mask.astype(np.float32) - 1.0) * 30.0)      # (B, S), 0 or -30
    pen_h = np.ascontiguousarray(
        penalty.reshape(B, 16, 128).transpose(2, 1, 0).reshape(128, 64))
    q_l = np.arange(512)
    maskt = np.empty((4, 128, 512), np.float32)
    for mi in range(4):
        k_l = mi * 128 + np.arange(128)
        maskt[mi] = (k_l[:, None] <= q_l[None, :]).astype(np.float32)
    maskt_h = np.ascontiguousarray(
        maskt.transpose(1, 0, 2).reshape(128, 2048))

    in2 = []
    for h in range(H):
        d0 = D * h
        in2.append({
            "qt": np.ascontiguousarray(QT[d0:d0 + D]),
            "kt": np.ascontiguousarray(KT[d0:d0 + D]),
            "vaug": _pad_vaug(V[:, d0:d0 + D]),
            "biasT": np.ascontiguousarray(position_bias[h].T),
            "wot": np.ascontiguousarray(Wo[:, d0:d0 + D].T),
            "pen": pen_h, "maskt": maskt_h,
        })
    res2 = _run(_get("attn", _build_attn), in2, "attn")

    attn = np.empty((B, H, S, S), np.float32)
    out = np.zeros((B, S, E), np.float32)
    for h in range(H):
        attn[:, h] = res2[h]["attn_out"]
        out += res2[h]["outp"]
    out += bo
    return out, attn


# revision 19
# speedup vs baseline: 1.0954x; 1.0954x over previous
"""Trainium2 Bass kernel for nn_MultiHeadAttention (B=4, S=2048, E=512, H=8).

Returns (out, attn) matching the reference:
    Q = q @ Wq.T + bq ... scores = QK^T/sqrt(D) + position_bias, causal+key mask,
    attn = softmax(scores), ctx = attn @ V, out = ctx @ Wo.T + bo.

Distribution over 8 NeuronCores, two SPMD launches:
  launch 1 (row-parallel): QKV projections; each core handles 1024 of the
      8192 flattened (B*S) rows, writing Q^T/K^T (head-major transposed) and
      V (natural) layouts.
  launch 2 (head-parallel): core h computes head h for all batches: scores
      are built transposed (k on partitions, q free) so softmax sums come
      free out of a V-augmented matmul and attn@V needs no transposes; the
      natural-layout attention output is produced by PE transposes; the
      output projection partial (summed over heads on host) is also done
      on device.

All matmuls run in float32r (TF32-like, ~1.5e-4 rel err; full PE rate).
"""

import numpy as np
import ml_dtypes

import concourse.bacc as bacc
import concourse.mybir as mybir
import concourse.tile as tile
from concourse.bass_utils import run_bass_kernel_spmd
from concourse.masks import make_identity

F32 = mybir.dt.float32
F32R = mybir.dt.float32r
BF16 = mybir.dt.bfloat16
EXP = mybir.ActivationFunctionType.Exp
IDENT = mybir.ActivationFunctionType.Identity

B, S, E, H = 4, 2048, 512, 8
D = E // H          # 64
SB = B * S          # 8192 flattened rows
R = SB // 8         # 1024 rows per core (launch 1)
NCORES = 8

TRACE = False
LAST_EXEC_NS = {}

_cache = {}


# ---------------------------------------------------------------- launch 1
def _build_proj():
    nc = bacc.Bacc("TRN2", target_bir_lowering=False, debug=False,
                   num_devices=NCORES)
    xq = nc.declare_dram_parameter("xq", [R, E], F32R, isOutput=False)
    xk = nc.declare_dram_parameter("xk", [R, E], F32R, isOutput=False)
    xv = nc.declare_dram_parameter("xv", [R, E], F32R, isOutput=False)
    wqt = nc.declare_dram_parameter("wqt", [E, E], F32R, isOutput=False)
    wkt = nc.declare_dram_parameter("wkt", [E, E], F32R, isOutput=False)
    wvt = nc.declare_dram_parameter("wvt", [E, E], F32R, isOutput=False)
    bq2 = nc.declare_dram_parameter("bq2", [128, 4], F32, isOutput=False)
    bk2 = nc.declare_dram_parameter("bk2", [128, 4], F32, isOutput=False)
    bvrow = nc.declare_dram_parameter("bvrow", [1, E], F32R, isOutput=False)
    qt_out = nc.declare_dram_parameter("qt_out", [E, R], F32, isOutput=True)
    kt_out = nc.declare_dram_parameter("kt_out", [E, R], F32, isOutput=True)
    v_out = nc.declare_dram_parameter("v_out", [R, E], F32, isOutput=True)

    NT = R // 128   # 8 row tiles
    NE = E // 128   # 4 emb chunks

    with tile.TileContext(nc) as tc:
        with tc.tile_pool(name="const", bufs=1) as const, \
             tc.tile_pool(name="w", bufs=1) as wpool, \
             tc.tile_pool(name="xt", bufs=1) as xtp, \
             tc.tile_pool(name="ld", bufs=3) as ld, \
             tc.tile_pool(name="st", bufs=3) as st, \
             tc.tile_pool(name="tp", bufs=3, space="PSUM") as tps, \
             tc.tile_pool(name="mp", bufs=3, space="PSUM") as mps:
            ident = const.tile([128, 128], F32, tag="ident")
            make_identity(nc, ident)
            ident_r = const.tile([128, 128], F32R, tag="ident_r")
            nc.vector.tensor_copy(ident_r[:], ident[:])
            ones_f = const.tile([1, 128], F32, tag="ones_f")
            nc.vector.memset(ones_f[:], 1.0)
            ones_r = const.tile([1, 128], F32R, tag="ones_r")
            nc.vector.tensor_copy(ones_r[:], ones_f[:])
            bq_sb = const.tile([128, 4], F32, tag="bq")
            bk_sb = const.tile([128, 4], F32, tag="bk")
            bv_sb = const.tile([1, E], F32R, tag="bv")
            nc.sync.dma_start(bq_sb[:], bq2[:])
            nc.sync.dma_start(bk_sb[:], bk2[:])
            nc.sync.dma_start(bv_sb[:], bvrow[:])

            ws = {}
            for name, wt in (("q", wqt), ("k", wkt), ("v", wvt)):
                ws[name] = [wpool.tile([128, E], F32R, tag=f"w{name}{i}",
                                       name=f"w{name}{i}")
                            for i in range(NE)]
                for i in range(NE):
                    nc.sync.dma_start(ws[name][i][:], wt[128 * i:128 * (i + 1), :])

            # transposed inputs: xT[name][e] is (128, R) slab of x^T
            xT = {}
            for name, x in (("q", xq), ("k", xk), ("v", xv)):
                xT[name] = [xtp.tile([128, R], F32R, tag=f"xT{name}{e}",
                                     name=f"xT{name}{e}")
                            for e in range(NE)]
                for t in range(NT):
                    xt_ld = ld.tile([128, E], F32R, tag="xld")
                    nc.sync.dma_start(xt_ld[:], x[128 * t:128 * (t + 1), :])
                    for e in range(NE):
                        pt = tps.tile([128, 128], F32R, tag="tp")
                        nc.tensor.matmul(pt[:], xt_ld[:, 128 * e:128 * (e + 1)],
                                         ident_r[:], is_transpose=True,
                                         start=True, stop=True)
                        nc.vector.tensor_copy(
                            xT[name][e][:, 128 * t:128 * (t + 1)], pt[:])

            # Q^T / K^T : out[e_out block, row chunk] = sum_e_in w[e_in][:, e_out].T @ xT[e_in]
            for name, out_t, bias_sb in (("q", qt_out, bq_sb), ("k", kt_out, bk_sb)):
                for eo in range(NE):
                    for rt in range(R // 512):
                        ps = mps.tile([128, 512], F32, tag="mm")
                        for ei in range(NE):
                            nc.tensor.matmul(
                                ps[:],
                                ws[name][ei][:, 128 * eo:128 * (eo + 1)],
                                xT[name][ei][:, 512 * rt:512 * (rt + 1)],
                                start=(ei == 0), stop=(ei == NE - 1))
                        ot = st.tile([128, 512], F32, tag="qkst")
                        nc.scalar.activation(ot[:], ps[:], IDENT,
                                             bias=bias_sb[:, eo:eo + 1], scale=1.0)
                        nc.sync.dma_start(
                            out_t[128 * eo:128 * (eo + 1), 512 * rt:512 * (rt + 1)],
                            ot[:])

            # V natural: out[row tile, :] = x[rows] @ wvt + bv
            for t in range(NT):
                ps = mps.tile([128, 512], F32, tag="mm")
                for ei in range(NE):
                    nc.tensor.matmul(ps[:], xT["v"][ei][:, 128 * t:128 * (t + 1)],
                                     ws["v"][ei][:], start=(ei == 0), stop=False)
                nc.tensor.matmul(ps[:], ones_r[:], bv_sb[:], start=False, stop=True)
                ot = st.tile([128, 512], F32, tag="vst")
                nc.vector.tensor_copy(ot[:], ps[:])
                nc.sync.dma_start(v_out[128 * t:128 * (t + 1), :], ot[:])
    nc.compile()
    return nc


# ---------------------------------------------------------------- launch 2
def _build_attn():
    import os
    dbg_nc = int(os.environ.get("KER_NC", "4"))
    dbg_nb = int(os.environ.get("KER_NB", "4"))
    dbg_noattn = os.environ.get("KER_NOATTN", "0") == "1"
    dbg_nozero = os.environ.get("KER_NOZERO", "0") == "1"
    dbg_nonat = os.environ.get("KER_NONAT", "0") == "1"
    nc = bacc.Bacc("TRN2", target_bir_lowering=False, debug=False,
                   num_devices=NCORES)
    qt = nc.declare_dram_parameter("qt", [D, SB], F32R, isOutput=False)
    kt = nc.declare_dram_parameter("kt", [D, SB], F32R, isOutput=False)
    # V padded to 128 cols: [:, :64] = V_h, [:, 64] = 1 (row sums), rest 0.
    # fp32r matmuls need all 128 PE col groups active (output partitions).
    vaug = nc.declare_dram_parameter("vaug", [SB, 128], F32R, isOutput=False)
    biasT = nc.declare_dram_parameter("biasT", [S, S], BF16, isOutput=False)
    wot = nc.declare_dram_parameter("wot", [D, E], F32R, isOutput=False)
    pen = nc.declare_dram_parameter("pen", [128, 64], F32, isOutput=False)
    maskt = nc.declare_dram_parameter("maskt", [128, 2048], F32, isOutput=False)
    attn_out = nc.declare_dram_parameter("attn_out", [B, S, S], F32, isOutput=True)
    outp = nc.declare_dram_parameter("outp", [B, S, E], F32, isOutput=True)

    NKT = S // 128      # 16 k tiles per batch
    NC_ = S // 512      # 4 q chunks per batch

    with tile.TileContext(nc) as tc:
        with tc.tile_pool(name="const", bufs=1) as const, \
             tc.tile_pool(name="bias", bufs=2) as pb, \
             tc.tile_pool(name="expp", bufs=2) as ep, \
             tc.tile_pool(name="astage", bufs=1) as ap_, \
             tc.tile_pool(name="ostage", bufs=2) as op_, \
             tc.tile_pool(name="small", bufs=2) as sp_, \
             tc.tile_pool(name="scp", bufs=2, space="PSUM") as scp, \
             tc.tile_pool(name="ctxp", bufs=2, space="PSUM") as ctxp, \
             tc.tile_pool(name="natp", bufs=2, space="PSUM") as natp_, \
             tc.tile_pool(name="opp", bufs=1, space="PSUM") as opp_, \
             tc.tile_pool(name="sump", bufs=1, space="PSUM") as sump:
            ident = const.tile([128, 128], F32, tag="ident")
            make_identity(nc, ident)
            ident_r = const.tile([128, 128], F32R, tag="ident_r")
            nc.vector.tensor_copy(ident_r[:], ident[:])
            ident_b = const.tile([128, 128], BF16, tag="ident_b")
            nc.vector.tensor_copy(ident_b[:], ident[:])

            kt_sb = const.tile([D, SB], F32R, tag="kt")
            for i in range(4):
                # single (64, 8192) DMAs crash the device; chunk to (64, 2048)
                nc.sync.dma_start(kt_sb[:, 2048 * i:2048 * (i + 1)],
                                  kt[:, 2048 * i:2048 * (i + 1)])
            vaug_sb = const.tile([128, (SB // 128) * 128], F32R, tag="vaug")
            for t0 in range(0, SB // 128, 8):
                nc.sync.dma_start(
                    vaug_sb[:, 128 * t0:128 * (t0 + 8)].rearrange(
                        "p (t d) -> p t d", d=128),
                    vaug[128 * t0:128 * (t0 + 8), :].rearrange(
                        "(t p) d -> p t d", p=128))
            wot_sb = const.tile([D, E], F32R, tag="wot")
            nc.sync.dma_start(wot_sb[:], wot[:])
            pen_sb = const.tile([128, 64], F32, tag="pen")
            nc.sync.dma_start(pen_sb[:], pen[:])
            maskt_sb = const.tile([128, 2048], F32, tag="maskt")
            nc.sync.dma_start(maskt_sb[:], maskt[:])
            for _zi in range(2):
                zt = ap_.tile([128, 2048], F32, tag="astage", name=f"az{_zi}")
                nc.vector.memset(zt[:], 0.0)

            for c in range(dbg_nc):
                jn = 4 * c + 4   # k tiles in this chunk column
                # bias^T tiles for this q-chunk, shared across batches
                bts = []
                for j in range(jn):
                    bt = pb.tile([128, 512], BF16, tag=f"bias{j}",
                                 name=f"bias{j}_{c}")
                    nc.sync.dma_start(
                        bt[:], biasT[128 * j:128 * (j + 1), 512 * c:512 * (c + 1)])
                    bts.append(bt)
                for b in range(dbg_nb):
                    qtc = sp_.tile([D, 512], F32R, tag="qtc", bufs=3,
                                   name=f"qtc{c}_{b}")
                    nc.sync.dma_start(
                        qtc[:], qt[:, S * b + 512 * c:S * b + 512 * (c + 1)])
                    ctps = ctxp.tile([128, 512], F32, tag="ctx")
                    expT = []
                    for j in range(jn):
                        sc = scp.tile([128, 512], F32, tag="sc")
                        nc.tensor.matmul(
                            sc[:],
                            kt_sb[:, S * b + 128 * j:S * b + 128 * (j + 1)],
                            qtc[:],
                            start=True, stop=False)
                        nc.tensor.matmul(sc[:], ident_b[:], bts[j][:],
                                         start=False, stop=True,
                                         skip_group_check=True)
                        e = ep.tile([128, 512], F32R, tag=f"expT{j}")
                        nc.scalar.activation(e[:], sc[:], EXP,
                                             bias=pen_sb[:, 4 * j + b:4 * j + b + 1],
                                             scale=1.0)
                        if j >= 4 * c:
                            nc.vector.tensor_mul(
                                e[:], e[:],
                                maskt_sb[:, 512 * (j - 4 * c):512 * (j - 4 * c + 1)])
                        nc.tensor.matmul(
                            ctps[:],
                            vaug_sb[:, (16 * b + j) * 128:(16 * b + j + 1) * 128],
                            e[:], start=(j == 0), stop=(j == jn - 1),
                            skip_group_check=True)
                        expT.append(e)
                    # evict ctx^T rows [0..64) for the output projection,
                    # and rows [64..66) (sums + junk) shifted to partition 0
                    # for the 2-row PE transpose (fp32r needs an even count;
                    # non-zero base-partition transposes misbehave).
                    cts = sp_.tile([D, 512], F32R, tag="cts")
                    nc.vector.tensor_copy(cts[:], ctps[0:D, :])
                    s2 = sp_.tile([2, 512], F32R, tag="s2")
                    nc.vector.tensor_copy(s2[:], ctps[D:D + 2, :])
                    scol = sump.tile([128, 8], F32R, tag="scol")
                    for qm in range(4):
                        nc.tensor.matmul(scol[:, 2 * qm:2 * qm + 2],
                                         s2[:, 128 * qm:128 * (qm + 1)],
                                         ident_r[0:2, 0:2],
                                         is_transpose=True,
                                         start=True, stop=True)
                    rc = sp_.tile([128, 8], F32, tag="rc")
                    nc.vector.reciprocal(rc[:], scol[:])
                    for qm in range(4):
                        # output projection partial (normalized at evict)
                        op = opp_.tile([128, 512], F32, tag="op")
                        nc.tensor.matmul(op[:],
                                         cts[:, 128 * qm:128 * (qm + 1)],
                                         wot_sb[:], start=True, stop=True)
                        os = op_.tile([128, 512], F32, tag="os")
                        nc.scalar.mul(os[:], op[:], rc[:, 2 * qm:2 * qm + 1])
                        r0 = 512 * c + 128 * qm
                        nc.sync.dma_start(outp[b, r0:r0 + 128, :], os[:])
                        if dbg_noattn:
                            continue
                        # natural-layout attention rows
                        ast = ap_.tile([128, 2048], F32, tag="astage")
                        if not dbg_nonat:
                            for jg in range(c + 1):
                                np_t = natp_.tile([128, 512], F32R, tag="nat")
                                for jj in range(4):
                                    j = 4 * jg + jj
                                    nc.tensor.matmul(
                                        np_t[:, 128 * jj:128 * (jj + 1)],
                                        expT[j][:, 128 * qm:128 * (qm + 1)],
                                        ident_r[:], is_transpose=True,
                                        start=True, stop=True)
                                nc.vector.tensor_scalar_mul(
                                    ast[:, 512 * jg:512 * (jg + 1)], np_t[:],
                                    rc[:, 2 * qm:2 * qm + 1])
                        else:
                            nc.vector.memset(ast[:, 0:512 * (c + 1)], 0.5)
                        # slots pre-zeroed at start; tail shrinks as c grows,
                        # so [512(c+1):2048] always holds causal zeros
                        nc.sync.dma_start(attn_out[b, r0:r0 + 128, :], ast[:])
    nc.compile()
    return nc


# ---------------------------------------------------------------- host glue
def _get(name, builder):
    if name not in _cache:
        _cache[name] = builder()
    return _cache[name]


def _run(nc, in_maps, label):
    res = run_bass_kernel_spmd(nc, in_maps, core_ids=list(range(NCORES)),
                               trace=TRACE)
    if TRACE:
        LAST_EXEC_NS[label] = res.exec_time_ns
    return res.results


def _pad_vaug(v):
    out = np.zeros((SB, 128), np.float32)
    out[:, :D] = v
    out[:, D] = 1.0
    return out


def kernel(query, key, value, mask, position_bias,
           Wq, bq, Wk, bk, Wv, bv, Wo, bo):
    query = np.asarray(query, np.float32).reshape(SB, E)
    key_ = np.asarray(key, np.float32).reshape(SB, E)
    value = np.asarray(value, np.float32).reshape(SB, E)
    mask = np.asarray(mask)
    position_bias = np.asarray(position_bias, np.float32)
    Wq = np.asarray(Wq, np.float32); bq = np.asarray(bq, np.float32)
    Wk = np.asarray(Wk, np.float32); bk = np.asarray(bk, np.float32)
    Wv = np.asarray(Wv, np.float32); bv = np.asarray(bv, np.float32)
    Wo = np.asarray(Wo, np.float32); bo = np.asarray(bo, np.float32)

    scale = 1.0 / np.sqrt(D).astype(np.float32)

    # ---- launch 1: projections, row-sharded
    wqt = np.ascontiguousarray(Wq.T * scale)
    wkt = np.ascontiguousarray(Wk.T)
    wvt = np.ascontiguousarray(Wv.T)
    bq2 = np.ascontiguousarray((bq * scale).reshape(4, 128).T)
    bk2 = np.ascontiguousarray(bk.reshape(4, 128).T)
    bvrow = np.ascontiguousarray(bv.reshape(1, E))
    in1 = []
    for c in range(NCORES):
        r0 = R * c
        in1.append({
            "xq": query[r0:r0 + R], "xk": key_[r0:r0 + R], "xv": value[r0:r0 + R],
            "wqt": wqt, "wkt": wkt, "wvt": wvt,
            "bq2": bq2, "bk2": bk2, "bvrow": bvrow,
        })
    res1 = _run(_get("proj", _build_proj), in1, "proj")

    QT = np.empty((E, SB), np.float32)
    KT = np.empty((E, SB), np.float32)
    V = np.empty((SB, E), np.float32)
    for c in range(NCORES):
        QT[:, R * c:R * (c + 1)] = res1[c]["qt_out"]
        KT[:, R * c:R * (c + 1)] = res1[c]["kt_out"]
        V[R * c:R * (c + 1)] = res1[c]["v_out"]

    # ---- launch 2: attention, head-parallel
    penalty = ((mask.astype(np.float32) - 1.0) * 30.0)      # (B, S), 0 or -30
    pen_h = np.ascontiguousarray(
        penalty.reshape(B, 16, 128).transpose(2, 1, 0).reshape(128, 64))
    q_l = np.arange(512)
    maskt = np.empty((4, 128, 512), np.float32)
    for mi in range(4):
        k_l = mi * 128 + np.arange(128)
        maskt[mi] = (k_l[:, None] <= q_l[None, :]).astype(np.float32)
    maskt_h = np.ascontiguousarray(
        maskt.transpose(1, 0, 2).reshape(128, 2048))

    in2 = []
    for h in range(H):
        d0 = D * h
        in2.append({
            "qt": np.ascontiguousarray(QT[d0:d0 + D]),
            "kt": np.ascontiguousarray(KT[d0:d0 + D]),
            "vaug": _pad_vaug(V[:, d0:d0 + D]),
            "biasT": np.ascontiguousarray(
                position_bias[h].T.astype(ml_dtypes.bfloat16)),
            "wot": np.ascontiguousarray(Wo[:, d0:d0 + D].T),
            "pen": pen_h, "maskt": maskt_h,
        })
    res2 = _run(_get("attn", _build_attn), in2, "attn")

    attn = np.empty((B, H, S, S), np.float32)
    out = np.zeros((B, S, E), np.float32)
    for h in range(H):
        attn[:, h] = res2[h]["attn_out"]
        out += res2[h]["outp"]
    out += bo
    return out, attn
